# revision 24
# baseline (speedup 1.0000x reference)
"""Block floating-point quantizer (DMX BFP: PRECISION=8, BLOCK_SIZE=128) on 8
Trainium2 NeuronCores.

Math (per 128-elem block along the last dim):
    amax  = max(|x_block|)
    e     = floor(log2(amax))            (guarded for amax == 0)
    scale = 2^(e-6)
    y     = clip(round_half_even(x/scale), -127, 127) * scale

Implemented without division/log/exp via exact fp32 tricks:
    p2  = 2^e, recovered exactly by masking the fp32 exponent field of amax
    C   = 1.5*2^23*scale = p2 * 196608.0          (exact product)
    hs2 = pred(127.5*scale) = p2 * pred(1.9921875) (exact product)
    xc  = clamp(x, -hs2, hs2)   -- pre-clamp; equivalent to post-clamping q
          to [-127, 127] because pred(127.5*scale) rounds to 127*scale and
          no float lies in (pred(127.5*scale), 127.5*scale)
    y   = (xc + C) - C          -- fp32 RNE add rounds xc to a multiple of
          scale (ulp(t) == scale inside [2^23*scale, 2^24*scale)); the
          subtract is exact (Sterbenz-adjacent: result is representable)

The pre-clamp bound does not need to be exact: any b in
(126.5*scale, 127.5*scale) yields identical output (everything at or
beyond it quantizes to q=127), so b = C*K3 with K3 ~= 127/(1.5*2^23) is
derived in-instruction from the C stream.

Each core handles a 1024-row shard. Per [128, 4096] tile: one abs-max
tensor_reduce, three tiny [128, 32] ops, then ONE fused custom-DVE
instruction (min, neg, max, add, sub in a single 8-stage pass) over the
whole tile -- per-block scalars are delivered as a stride-0-broadcast
Src1 stream, so instruction overhead amortizes over 4096 elements
instead of 128. Input DMAs ride the SP HWDGE ring; output DMAs ride
gpsimd SWDGE rings, avoiding head-of-line blocking between directions.
"""

import sys

for _p in ("/opt/trn_rl_repo",):
    if _p not in sys.path:
        sys.path.insert(0, _p)

import numpy as np

import concourse.bacc as bacc
import concourse.bass as bass
import concourse.tile as tile
from concourse import mybir
from concourse import dve_ops as _dve_ops
from concourse.bass_utils import run_bass_kernel_spmd
from concourse.dve_ops import DveOp, has_src1
from concourse.dve_spec import C1, Spec, Src0, Src1, Zero, maxx, minn
from concourse.dve_spec import lower as _dve_lower
from concourse.dve_uop import DveOpSpec

N_CORES = 8
ROWS, COLS = 8192, 8192
SHARD_ROWS = ROWS // N_CORES  # 1024
P = 128                       # SBUF partitions
BLK = 128                     # shared-exponent block size
NBLK = COLS // BLK            # 64 blocks per row

EXP_MASK = 0x7F800000         # fp32 exponent-field mask
C_MULT = 196608.0             # 1.5 * 2^17: p2 * C_MULT == 1.5*2^23*scale, exact
K3 = float(np.float32(127.0 / (1.5 * 2**23)))
#                             # C*K3 ~= 127*scale, well inside (126.5, 127.5)*scale

_F32 = mybir.dt.float32
_I32 = mybir.dt.int32


def _register_dve_op(name, spec):
    """Register a custom DVE op in the module-level tables at runtime
    (same three structures dve_ops.py populates at import)."""
    for op in _dve_ops.OPS:
        if op.name == name:
            return op
    row = _dve_ops._CUSTOM_DVE_ROW_BASE + len(_dve_ops.OPS)
    assert row < 0x20, "custom-DVE row field overflow"
    _dve_ops._SUB_OPCODE_FOR_NAME[name] = row
    shas = {}
    for ver in ("v3", "v4"):
        uops = _dve_lower(spec, ver=ver)
        shas[ver] = DveOpSpec(
            name=name, opcode=row, uops=uops, rd1_en=has_src1(spec)
        ).sha(ver)
    op = DveOp(name, spec, subdim=False, uops_sha=shas)
    _dve_ops.OPS.append(op)
    _dve_ops.CUSTOM_DVE_SPECS[name] = spec
    return op


def _match(in0, in1):
    # CoreSim may hand in0 as the coalesced view while in1 keeps its
    # 3D broadcast shape; reconcile to in1's shape (same element order).
    if in1 is not None and in0.shape != in1.shape:
        in0 = in0.reshape(in1.shape)
    return in0, in1


def _quant_ref(in0, in1, c0, c1, c2):
    # in0 = x, in1 = C stream, c1 = K3. fp32 throughout:
    #   b  = C*K3   (any value in (126.5, 127.5)*scale is correct)
    #   xc = clamp(x, -b, b); y = (xc + C) - C  (RNE between the ops)
    in0, in1 = _match(in0, in1)
    f32 = np.float32
    b = (in1 * f32(c1)).astype(f32)
    xc = np.maximum(np.minimum(in0, b), (f32(0.0) - b).astype(f32))
    t = (xc + in1).astype(f32)
    return (t - in1).astype(f32)


_m1 = Src1 * C1
BFP_QUANT = _register_dve_op(
    "BFP_QUANT_ANT",
    Spec(
        body=(maxx(minn(Src0, _m1), Zero - _m1) + Src1) - Src1,
        reference=_quant_ref,
    ),
)


def build(
    shard_rows=SHARD_ROWS,
    cols=COLS,
    tile_cols=4096,
    io_bufs=3,
    alt_in=False,
    swq=2,
    yt_bufs=None,
):
    nblk_t = tile_cols // BLK
    col_tiles = cols // tile_cols
    if yt_bufs is None:
        yt_bufs = io_bufs
    nc = bacc.Bacc("TRN2", target_bir_lowering=False, num_swdge_queues=swq)
    x = nc.declare_dram_parameter("x", [shard_rows, cols], _F32, isOutput=False)
    y = nc.declare_dram_parameter("out", [shard_rows, cols], _F32, isOutput=True)

    with tile.TileContext(nc) as tc:
        with (
            tc.tile_pool(name="io", bufs=io_bufs) as io_pool,
            tc.tile_pool(name="oy", bufs=yt_bufs) as oy_pool,
            tc.tile_pool(name="small", bufs=3) as small_pool,
            tc.tile_pool(name="const", bufs=1) as const_pool,
        ):
            mask_c = const_pool.tile([P, 1], _I32, tag="mask")
            nc.vector.memset(mask_c[:], EXP_MASK)

            # Work items (row_tile, col_off, width). The first and last row
            # tiles are split into narrow chunks: a short first in-DMA lets
            # compute start sooner, a short last out-DMA shrinks the tail.
            row_tiles = shard_rows // P
            edge_w = min(2048, tile_cols)
            work = []
            for it in range(row_tiles):
                if it in (0, row_tiles - 1) and row_tiles > 1 and tile_cols > edge_w:
                    for co in range(0, cols, edge_w):
                        work.append((it, co, edge_w))
                else:
                    for jt in range(col_tiles):
                        work.append((it, jt * tile_cols, tile_cols))

            n_tile = 0
            for it, co, w in work:
                rs = slice(it * P, (it + 1) * P)
                if True:
                    n_tile += 1
                    cs = slice(co, co + w)
                    nblk_t = w // BLK
                    xt = io_pool.tile([P, w], _F32, tag="xt")
                    # Input DMAs ride the SP HWDGE ring; output DMAs ride
                    # gpsimd SWDGE rings. Separate rings per direction avoid
                    # head-of-line blocking (an out-DMA waiting on compute
                    # would otherwise stall the next in-DMA queued behind it).
                    in_eng = nc.sync if (not alt_in or n_tile % 2 == 0) else nc.scalar
                    in_eng.dma_start(out=xt[:], in_=x[rs, cs])

                    amax = small_pool.tile([P, nblk_t], _F32, tag="amax")
                    nc.vector.tensor_reduce(
                        out=amax[:],
                        in_=xt[:].rearrange("p (b k) -> p b k", k=BLK),
                        axis=mybir.AxisListType.X,
                        op=mybir.AluOpType.max,
                        apply_absolute_value=True,
                    )
                    # Zero-block guard: any nonzero scale works; output is 0
                    # for an all-zero block either way.
                    nc.vector.tensor_scalar_max(amax[:], amax[:], 1e-30)

                    p2 = small_pool.tile([P, nblk_t], _I32, tag="p2")
                    cmag = small_pool.tile([P, nblk_t], _F32, tag="cmag")
                    # p2 = 2^floor(log2(amax)) exactly, via the exponent field.
                    nc.vector.tensor_tensor(
                        out=p2[:],
                        in0=amax[:].bitcast(_I32),
                        in1=mask_c[:].to_broadcast((P, nblk_t)),
                        op=mybir.AluOpType.bitwise_and,
                    )
                    p2f = p2[:].bitcast(_F32)
                    nc.vector.tensor_scalar_mul(cmag[:], p2f, C_MULT)

                    yt = oy_pool.tile([P, w], _F32, tag="yt")
                    x3 = xt[:].rearrange("p (b k) -> p b k", k=BLK)
                    y3 = yt[:].rearrange("p (b k) -> p b k", k=BLK)
                    c3 = cmag[:].unsqueeze(2).to_broadcast((P, nblk_t, BLK))
                    nc.vector._custom_dve(BFP_QUANT, out=y3, in0=x3, in1=c3, s1=K3)

                    nc.gpsimd.dma_start(out=y[rs, cs], in_=yt[:])

    nc.compile()
    return nc


_nc_cache = {}


def _get_nc():
    if "nc" not in _nc_cache:
        _nc_cache["nc"] = build()
    return _nc_cache["nc"]


def kernel(x):
    x = np.ascontiguousarray(np.asarray(x, dtype=np.float32))
    assert x.shape == (ROWS, COLS)
    nc = _get_nc()
    in_maps = [
        {"x": x[i * SHARD_ROWS : (i + 1) * SHARD_ROWS]} for i in range(N_CORES)
    ]
    res = run_bass_kernel_spmd(nc, in_maps, core_ids=list(range(N_CORES)))
    return np.concatenate([r["out"] for r in res.results], axis=0)


# revision 30
# speedup vs baseline: 1.2139x; 1.2139x over previous
"""Block floating-point quantizer (DMX BFP: PRECISION=8, BLOCK_SIZE=128) on 8
Trainium2 NeuronCores.

Math (per 128-elem block along the last dim):
    amax  = max(|x_block|)
    e     = floor(log2(amax))            (guarded for amax == 0)
    scale = 2^(e-6)
    y     = clip(round_half_even(x/scale), -127, 127) * scale

Implemented without division/log/exp via exact fp32 tricks:
    p2  = 2^e, recovered exactly by masking the fp32 exponent field of amax
    C   = 1.5*2^23*scale = p2 * 196608.0          (exact product)
    hs2 = pred(127.5*scale) = p2 * pred(1.9921875) (exact product)
    xc  = clamp(x, -hs2, hs2)   -- pre-clamp; equivalent to post-clamping q
          to [-127, 127] because pred(127.5*scale) rounds to 127*scale and
          no float lies in (pred(127.5*scale), 127.5*scale)
    y   = (xc + C) - C          -- fp32 RNE add rounds xc to a multiple of
          scale (ulp(t) == scale inside [2^23*scale, 2^24*scale)); the
          subtract is exact (Sterbenz-adjacent: result is representable)

The pre-clamp bound does not need to be exact: any b in
(126.5*scale, 127.5*scale) yields identical output (everything at or
beyond it quantizes to q=127), so b = C*K3 with K3 ~= 127/(1.5*2^23) is
derived in-instruction from the C stream.

Each core handles a 1024-row shard. Per [128, 4096] tile: one abs-max
tensor_reduce, three tiny [128, 32] ops, then ONE fused custom-DVE
instruction (min, neg, max, add, sub in a single 8-stage pass) over the
whole tile -- per-block scalars are delivered as a stride-0-broadcast
Src1 stream, so instruction overhead amortizes over 4096 elements
instead of 128. Input DMAs ride the SP HWDGE ring; output DMAs ride
gpsimd SWDGE rings, avoiding head-of-line blocking between directions.
"""

import sys

for _p in ("/opt/trn_rl_repo",):
    if _p not in sys.path:
        sys.path.insert(0, _p)

import numpy as np

import concourse.bacc as bacc
import concourse.tile as tile
from concourse import mybir
from concourse import dve_ops as _dve_ops
from concourse.bass_utils import run_bass_kernel_spmd
from concourse.dve_ops import DveOp, has_src1
from concourse.dve_spec import C1, Spec, Src0, Src1, Zero, maxx, minn
from concourse.dve_spec import lower as _dve_lower
from concourse.dve_uop import DveOpSpec

N_CORES = 8
ROWS, COLS = 8192, 8192
SHARD_ROWS = ROWS // N_CORES  # 1024
P = 128                       # SBUF partitions
BLK = 128                     # shared-exponent block size

EXP_MASK = 0x7F800000         # fp32 exponent-field mask
C_MULT = 196608.0             # 1.5 * 2^17: p2 * C_MULT == 1.5*2^23*scale, exact
K3 = float(np.float32(127.0 / (1.5 * 2**23)))
#                             # C*K3 ~= 127*scale, well inside (126.5, 127.5)*scale

_F32 = mybir.dt.float32
_I32 = mybir.dt.int32


def _register_dve_op(name, spec):
    """Register a custom DVE op in the module-level tables at runtime
    (same three structures dve_ops.py populates at import)."""
    for op in _dve_ops.OPS:
        if op.name == name:
            return op
    row = _dve_ops._CUSTOM_DVE_ROW_BASE + len(_dve_ops.OPS)
    assert row < 0x20, "custom-DVE row field overflow"
    _dve_ops._SUB_OPCODE_FOR_NAME[name] = row
    shas = {}
    for ver in ("v3", "v4"):
        uops = _dve_lower(spec, ver=ver)
        shas[ver] = DveOpSpec(
            name=name, opcode=row, uops=uops, rd1_en=has_src1(spec)
        ).sha(ver)
    op = DveOp(name, spec, subdim=False, uops_sha=shas)
    _dve_ops.OPS.append(op)
    _dve_ops.CUSTOM_DVE_SPECS[name] = spec
    return op


def _match(in0, in1):
    # CoreSim may hand in0 as the coalesced view while in1 keeps its
    # 3D broadcast shape; reconcile to in1's shape (same element order).
    if in1 is not None and in0.shape != in1.shape:
        in0 = in0.reshape(in1.shape)
    return in0, in1


def _quant_ref(in0, in1, c0, c1, c2):
    # in0 = x, in1 = C stream, c1 = K3. fp32 throughout:
    #   b  = C*K3   (any value in (126.5, 127.5)*scale is correct)
    #   xc = clamp(x, -b, b); y = (xc + C) - C  (RNE between the ops)
    in0, in1 = _match(in0, in1)
    f32 = np.float32
    b = (in1 * f32(c1)).astype(f32)
    xc = np.maximum(np.minimum(in0, b), (f32(0.0) - b).astype(f32))
    t = (xc + in1).astype(f32)
    return (t - in1).astype(f32)


_m1 = Src1 * C1
BFP_QUANT = _register_dve_op(
    "BFP_QUANT_ANT",
    Spec(
        body=(maxx(minn(Src0, _m1), Zero - _m1) + Src1) - Src1,
        reference=_quant_ref,
    ),
)


def build(
    shard_rows=SHARD_ROWS,
    cols=COLS,
    tile_cols=4096,
    io_bufs=4,
    swq=2,
    yt_bufs=3,
):
    tile_cols = min(tile_cols, cols)
    col_tiles = cols // tile_cols
    if yt_bufs is None:
        yt_bufs = io_bufs
    nc = bacc.Bacc("TRN2", target_bir_lowering=False, num_swdge_queues=swq)
    x = nc.declare_dram_parameter("x", [shard_rows, cols], _F32, isOutput=False)
    y = nc.declare_dram_parameter("out", [shard_rows, cols], _F32, isOutput=True)

    with tile.TileContext(nc) as tc:
        with (
            tc.tile_pool(name="io", bufs=io_bufs) as io_pool,
            tc.tile_pool(name="oy", bufs=yt_bufs) as oy_pool,
            tc.tile_pool(name="small", bufs=3) as small_pool,
            tc.tile_pool(name="const", bufs=1) as const_pool,
        ):
            mask_c = const_pool.tile([P, 1], _I32, tag="mask")
            nc.vector.memset(mask_c[:], EXP_MASK)

            row_tiles = shard_rows // P
            work = [
                (it, jt * tile_cols, tile_cols)
                for it in range(row_tiles)
                for jt in range(col_tiles)
            ]

            for it, co, w in work:
                rs = slice(it * P, (it + 1) * P)
                cs = slice(co, co + w)
                nblk_t = w // BLK
                xt = io_pool.tile([P, w], _F32, tag="xt")
                # Input DMAs ride the SP HWDGE ring; output DMAs ride
                # gpsimd SWDGE rings. Separate rings per direction avoid
                # head-of-line blocking (an out-DMA waiting on compute
                # would otherwise stall the next in-DMA queued behind it).
                nc.sync.dma_start(out=xt[:], in_=x[rs, cs])

                amax = small_pool.tile([P, nblk_t], _F32, tag="amax")
                nc.vector.tensor_reduce(
                    out=amax[:],
                    in_=xt[:].rearrange("p (b k) -> p b k", k=BLK),
                    axis=mybir.AxisListType.X,
                    op=mybir.AluOpType.max,
                    apply_absolute_value=True,
                )
                # Zero-block guard: any nonzero scale works; output is 0
                # for an all-zero block either way.
                nc.vector.tensor_scalar_max(amax[:], amax[:], 1e-30)

                p2 = small_pool.tile([P, nblk_t], _I32, tag="p2")
                cmag = small_pool.tile([P, nblk_t], _F32, tag="cmag")
                # p2 = 2^floor(log2(amax)) exactly, via the exponent field.
                nc.vector.tensor_tensor(
                    out=p2[:],
                    in0=amax[:].bitcast(_I32),
                    in1=mask_c[:].to_broadcast((P, nblk_t)),
                    op=mybir.AluOpType.bitwise_and,
                )
                p2f = p2[:].bitcast(_F32)
                nc.vector.tensor_scalar_mul(cmag[:], p2f, C_MULT)

                yt = oy_pool.tile([P, w], _F32, tag="yt")
                x3 = xt[:].rearrange("p (b k) -> p b k", k=BLK)
                y3 = yt[:].rearrange("p (b k) -> p b k", k=BLK)
                c3 = cmag[:].unsqueeze(2).to_broadcast((P, nblk_t, BLK))
                nc.vector._custom_dve(BFP_QUANT, out=y3, in0=x3, in1=c3, s1=K3)

                nc.gpsimd.dma_start(out=y[rs, cs], in_=yt[:])

    nc.compile()
    return nc


_nc_cache = {}


def _get_nc():
    if "nc" not in _nc_cache:
        _nc_cache["nc"] = build()
    return _nc_cache["nc"]


def kernel(x):
    x = np.ascontiguousarray(np.asarray(x, dtype=np.float32))
    assert x.shape == (ROWS, COLS)
    nc = _get_nc()
    in_maps = [
        {"x": x[i * SHARD_ROWS : (i + 1) * SHARD_ROWS]} for i in range(N_CORES)
    ]
    res = run_bass_kernel_spmd(nc, in_maps, core_ids=list(range(N_CORES)))
    return np.concatenate([r["out"] for r in res.results], axis=0)
